# revision 1
# baseline (speedup 1.0000x reference)
"""Multi-head attention Trainium2 kernel (B=4, T=1024, C=1024, H=16, D=64).

Sharding over 8 NeuronCores: core c handles batch b = c//2 and head group
g = c%2 (heads [8g, 8g+8)).  Each core computes a partial out-projection
(its 8 heads' contribution, [T, C]); the host sums the two partials per
batch and adds b_out.  No on-device collectives.

Math (per core, fp32r matmuls, fp32 accumulation):
  XT = x[b].T (host-provided, [C, T])
  QT/KT[f, t] = Wqk[:, f].T @ XT       (pair-stacked [128, T] tiles, Q pre-
                                        scaled by 1/8 with bias folded in)
  V[t, f]     = XT-chunk.T @ Wv        (natural layout, ones col appended)
  S^T[k, q]   = KT-slice.T @ QT-slice  (per head, causal blocks only)
  P           = exp(S^T)  (ACT, valid region), tri mask on diagonal blocks
  vals^T/s    = [V | 1].T @ P          (s = denominator, free in row 64)
  out[q, c]   = vals^T.T @ Wout-slice  (normalized + bias on the way)
"""

import os
import numpy as np

import concourse.bass as bass
import concourse.mybir as mybir
import concourse.tile as tile
from concourse import bacc
from concourse.bass_utils import run_bass_kernel_spmd

B, T, C, H, D = 4, 1024, 1024, 16, 64
P = 128            # partitions
HPC = 8            # heads per core
PAIRS = 4          # head pairs per core
KT_TILES = T // P  # 8 k-tiles over sequence
QC = 512           # q-chunk (PSUM bank free size, fp32)
NQC = T // QC      # 2 q-chunks
F32 = mybir.dt.float32
F32R = mybir.dt.float32r
AF = mybir.ActivationFunctionType
ALU = mybir.AluOpType

_CACHE = {}
DEBUG_TAPS = bool(os.environ.get("KERNEL_DEBUG_TAPS"))


def _build_nc():
    nc = bacc.Bacc(None, target_bir_lowering=False)

    xT = nc.dram_tensor("xT", [C, T], F32R, kind="ExternalInput")
    wqk = nc.dram_tensor("wqk", [8, C, P], F32R, kind="ExternalInput")
    wv = nc.dram_tensor("wv", [C, HPC * D], F32R, kind="ExternalInput")
    wout = nc.dram_tensor("wout", [HPC * D, C], F32R, kind="ExternalInput")
    bqk = nc.dram_tensor("bqk", [P, 8], F32, kind="ExternalInput")
    bv = nc.dram_tensor("bv", [P, PAIRS], F32, kind="ExternalInput")
    tri = nc.dram_tensor("tri", [P, P], F32R, kind="ExternalInput")
    out = nc.dram_tensor("out", [T, C], F32, kind="ExternalOutput")
    if DEBUG_TAPS:
        dbg_s = nc.dram_tensor("dbg_s", [2, QC], F32, kind="ExternalOutput")
        dbg_r = nc.dram_tensor("dbg_r", [2, D, QC], F32, kind="ExternalOutput")
        dbg_vals = nc.dram_tensor("dbg_vals", [P, QC], F32, kind="ExternalOutput")
        dbg_p = nc.dram_tensor("dbg_p", [P, 2, QC], F32, kind="ExternalOutput")

    xT_r = xT.rearrange("(ko p) t -> p ko t", p=P)
    wv_r = wv.rearrange("(ko p) f -> p ko f", p=P)
    wout_r = wout.rearrange("(po p) c -> p po c", p=P)

    with tile.TileContext(nc) as tc:
        with (
            tc.tile_pool(name="consts", bufs=1) as consts,
            tc.tile_pool(name="xt", bufs=1) as xt_pool,
            tc.tile_pool(name="wqk_p", bufs=4) as wqk_pool,
            tc.tile_pool(name="qkt", bufs=8) as qkt_pool,
            tc.tile_pool(name="vsb", bufs=8) as v_pool,
            tc.tile_pool(name="probs", bufs=8) as p_pool,
            tc.tile_pool(name="vals", bufs=8) as vals_pool,
            tc.tile_pool(name="smal", bufs=4) as small_pool,
            tc.tile_pool(name="outs", bufs=3) as out_pool,
        ):
            # ---- constants (tiny, first in queue) ----
            bqk_sb = consts.tile([P, 8], F32)
            nc.gpsimd.dma_start(bqk_sb, bqk[:, :])
            bv_sb = consts.tile([P, PAIRS], F32)
            nc.gpsimd.dma_start(bv_sb, bv[:, :])
            tri_sb = consts.tile([P, P], F32R)
            nc.gpsimd.dma_start(tri_sb, tri[:, :])
            ones_f = consts.tile([P, D], F32)
            nc.vector.memset(ones_f, 1.0)
            ones_r = consts.tile([P, D], F32R)
            nc.vector.tensor_copy(ones_r, ones_f)

            # ---- all input DMAs up front, dependency-optimal order ----
            # wqk slot 0 heads the SP queue, slot 1 heads the ACT queue, so
            # QT0/KT0 can start as soon as the first XT k-tiles arrive.
            w_sb = {}
            for slot in range(2):
                w_sb[slot] = wqk_pool.tile(
                    [P, C // P, P], F32R, tag="wqk", name=f"wqk{slot}"
                )
                eng = nc.sync if slot == 0 else nc.scalar
                eng.dma_start(
                    w_sb[slot], wqk[slot].rearrange("(ko p) f -> p ko f", p=P)
                )
            # XT k-tiles split across SP and ACT queues (parallel arrival).
            xt_sb = xt_pool.tile([P, C // P, T], F32R)
            for ki in range(C // P):
                eng = nc.sync if ki % 2 == 0 else nc.scalar
                eng.dma_start(xt_sb[:, ki, :], xT_r[:, ki, :])
            # wv per k-tile (SWDGE queue), wout behind it.
            wv_sb = consts.tile([P, C // P, HPC * D], F32R)
            for ki in range(C // P):
                nc.gpsimd.dma_start(wv_sb[:, ki, :], wv_r[:, ki, :])
            wout_sb = consts.tile([P, PAIRS, C], F32R)
            nc.gpsimd.dma_start(wout_sb, wout_r[:, :, :])
            for slot in range(2, 8):
                w_sb[slot] = wqk_pool.tile(
                    [P, C // P, P], F32R, tag="wqk", name=f"wqk{slot}"
                )
                nc.sync.dma_start(
                    w_sb[slot], wqk[slot].rearrange("(ko p) f -> p ko f", p=P)
                )

            # ---- QT/KT pair-stacked [128, T] ----
            qkv_ps_ctx = tc.tile_pool(name="qkv_ps", bufs=2, space="PSUM")
            qkv_ps = qkv_ps_ctx.__enter__()
            qt_sb = {}
            kt_sb = {}

            def emit_qkt(pair, kind):
                slot = 2 * pair + kind
                dst = qkt_pool.tile(
                    [P, T], F32R, tag="qkt", name=f"{'qk'[kind]}t{pair}"
                )
                ps = qkv_ps.tile([P, NQC, 512], F32, tag="qkv", name=f"qkvps{slot}")
                for qc in range(NQC):
                    for ki in range(C // P):
                        nc.tensor.matmul(
                            ps[:, qc, :],
                            w_sb[slot][:, ki, :],
                            xt_sb[:, ki, qc * QC : (qc + 1) * QC],
                            start=(ki == 0),
                            stop=(ki == C // P - 1),
                        )
                nc.scalar.activation(
                    dst,
                    ps.rearrange("p a b -> p (a b)"),
                    AF.Identity,
                    bias=bqk_sb[:, slot : slot + 1],
                    scale=0.125 if kind == 0 else 1.0,
                )
                return dst

            qt_sb[0] = emit_qkt(0, 0)
            kt_sb[0] = emit_qkt(0, 1)

            # ---- V natural layout [T-tile, head, D(+ones)] ----
            v_sb = []
            for ti in range(KT_TILES):
                vt = v_pool.tile([P, HPC, D + 1], F32R, tag="v_sb", name=f"v{ti}")
                v_sb.append(vt)
                ps = qkv_ps.tile([P, 512], F32, tag="qkv")
                for ki in range(C // P):
                    nc.tensor.matmul(
                        ps,
                        xt_sb[:, ki, ti * P : (ti + 1) * P],
                        wv_sb[:, ki, :],
                        start=(ki == 0),
                        stop=(ki == C // P - 1),
                    )
                nc.vector.tensor_copy(
                    vt[:, :, 0:D], ps.rearrange("p (h d) -> p h d", h=HPC)
                )
                nc.vector.tensor_copy(
                    vt[:, :, D : D + 1],
                    ones_r[:, 0:1].to_broadcast([P, HPC, 1]),
                )

            for pair in range(1, PAIRS):
                qt_sb[pair] = emit_qkt(pair, 0)
                kt_sb[pair] = emit_qkt(pair, 1)

            qkv_ps_ctx.__exit__(None, None, None)
            s_ps_ctx = tc.tile_pool(name="s_ps", bufs=2, space="PSUM")
            s_ps = s_ps_ctx.__enter__()
            v_ps_ctx = tc.tile_pool(name="v_ps", bufs=2, space="PSUM")
            v_ps = v_ps_ctx.__enter__()
            o_ps_ctx = tc.tile_pool(name="o_ps", bufs=2, space="PSUM")
            o_ps = o_ps_ctx.__enter__()

            # ---- attention ----
            vals_sb = {}  # (pair, qc) -> [P, QC]

            def emit_attention(pair, qc):
                """Both heads of the pair; score matmuls interleaved per k-tile
                so the two K=64 matmuls land in different PE row groups
                back-to-back (concurrent on HW)."""
                qt = qt_sb[pair]
                kt = kt_sb[pair]
                n_kt = 4 * (qc + 1)
                p_tiles = []
                for kj in range(n_kt):
                    j0 = kj - 4 * qc
                    q_lo = max(j0, 0) * P
                    pt = p_pool.tile([P, 2, QC], F32R, tag="probs")
                    p_tiles.append((pt, q_lo))
                    sps = s_ps.tile([P, 2, 512], F32, tag="s", name="sps")
                    for hl in range(2):
                        d0 = D * hl
                        nc.tensor.matmul(
                            sps[:, hl, q_lo:QC],
                            kt[d0 : d0 + D, kj * P : (kj + 1) * P],
                            qt[d0 : d0 + D, qc * QC + q_lo : (qc + 1) * QC],
                            start=True,
                            stop=True,
                        )
                    nc.scalar.activation(
                        pt[:, :, q_lo:QC], sps[:, :, q_lo:QC], AF.Exp
                    )
                    if j0 >= 0:
                        nc.gpsimd.tensor_tensor(
                            pt[:, :, q_lo : q_lo + P],
                            pt[:, :, q_lo : q_lo + P],
                            tri_sb[:, None, :].to_broadcast([P, 2, P]),
                            ALU.mult,
                        )
                key = (pair, qc)
                vals_sb[key] = vals_pool.tile(
                    [P, QC], F32R, tag="vals", name=f"vals{pair}_{qc}"
                )
                heads = []
                for hl in range(2):
                    h_abs = 2 * pair + hl
                    vps = v_ps.tile([P, 512], F32, tag="vps")
                    for kj in range(n_kt):
                        pt, q_lo = p_tiles[kj]
                        nc.tensor.matmul(
                            vps[0 : D + 1, q_lo:QC],
                            v_sb[kj][:, h_abs, :],
                            pt[:, hl, q_lo:QC],
                            start=(kj == 0),
                            stop=(kj == n_kt - 1),
                            skip_group_check=True,
                        )
                    # normalize part 1: s = row D of vps -> SBUF, then DMA
                    # partition-broadcast (both heads before the recips so the
                    # in-order DVE stream doesn't stall on DMA latency)
                    s_sb = small_pool.tile([P, QC], F32, tag="s_sb")
                    nc.vector.tensor_copy(s_sb[D : D + 1, :], vps[D : D + 1, :])
                    s_bc = small_pool.tile([D, QC], F32, tag="s_bc")
                    nc.sync.dma_start(
                        s_bc, s_sb[D : D + 1, None, :].to_broadcast([1, D, QC])
                    )
                    heads.append((vps, s_bc))
                for hl in range(2):
                    d0 = D * hl
                    vps, s_bc = heads[hl]
                    # normalize part 2: reciprocal at base 0 (custom-DVE ops
                    # require base_partition 0), multiply into vals.
                    r_sb = small_pool.tile([D, QC], F32, tag="r_sb")
                    nc.vector.reciprocal_approx_fast(r_sb, s_bc)
                    nc.vector.tensor_tensor(
                        vals_sb[key][d0 : d0 + D, :], vps[0:D, :], r_sb, ALU.mult
                    )
                    if DEBUG_TAPS and pair == 0 and qc == 0:
                        nc.sync.dma_start(dbg_s[hl : hl + 1, :], s_sb[D : D + 1, :])
                        nc.sync.dma_start(dbg_r[hl], r_sb)
                nc.vector.tensor_scalar_add(
                    vals_sb[key], vals_sb[key], bv_sb[:, pair : pair + 1]
                )
                if DEBUG_TAPS and pair == 0 and qc == 0:
                    nc.gpsimd.dma_start(dbg_vals[:, :], vals_sb[key])
                    nc.gpsimd.dma_start(dbg_p[:, :, :], p_tiles[0][0])

            for qc in range(NQC):
                for pair in range(PAIRS):
                    emit_attention(pair, qc)
            for qc in range(NQC):
                # ---- partial out-projection for this q-chunk ----
                for tsub in range(QC // P):
                    q0 = tsub * P
                    for cc in range(C // 512):
                        ops = o_ps.tile([P, 512], F32, tag="ops")
                        for pair in range(PAIRS):
                            nc.tensor.matmul(
                                ops,
                                vals_sb[(pair, qc)][:, q0 : q0 + P],
                                wout_sb[:, pair, cc * 512 : (cc + 1) * 512],
                                start=(pair == 0),
                                stop=(pair == PAIRS - 1),
                            )
                        o_sb = out_pool.tile([P, 512], F32, tag="o_sb")
                        if qc == 0:
                            nc.vector.tensor_copy(o_sb, ops)
                        else:
                            # tail window: ACT is idle, DVE is busy with the
                            # last normalize chains
                            nc.scalar.activation(o_sb, ops, AF.Copy)
                        nc.sync.dma_start(
                            out[qc * QC + q0 : qc * QC + q0 + P,
                                cc * 512 : (cc + 1) * 512],
                            o_sb,
                        )
            o_ps_ctx.__exit__(None, None, None)
            v_ps_ctx.__exit__(None, None, None)
            s_ps_ctx.__exit__(None, None, None)

    nc.compile()
    return nc


def _host_shards(x, mask, W_in, b_in, W_out, b_out):
    """Build the 8 per-core input maps."""
    del mask  # causal structure is hardcoded (tri tile built locally)
    x = np.asarray(x, dtype=np.float32)
    W_in = np.asarray(W_in, dtype=np.float32)
    b_in = np.asarray(b_in, dtype=np.float32)
    W_out = np.asarray(W_out, dtype=np.float32)

    tri = np.triu(np.ones((P, P), dtype=np.float32))  # tri[k, q] = 1 if k <= q
    xTs = [np.ascontiguousarray(x[b].T) for b in range(B)]

    per_group = {}
    for g in range(2):
        wqk = np.empty((8, C, P), dtype=np.float32)
        bqk = np.empty((P, 8), dtype=np.float32)
        for p in range(PAIRS):
            qcols = slice((8 * g + 2 * p) * D, (8 * g + 2 * p + 2) * D)
            kcols = slice(C + (8 * g + 2 * p) * D, C + (8 * g + 2 * p + 2) * D)
            wqk[2 * p] = W_in[:, qcols]  # scale 1/8 applied on-device (ACT copy)
            wqk[2 * p + 1] = W_in[:, kcols]
            bqk[:, 2 * p] = b_in[qcols] * 0.125
            bqk[:, 2 * p + 1] = b_in[kcols]
        vcols = slice(2 * C + g * 512, 2 * C + (g + 1) * 512)
        wv = np.ascontiguousarray(W_in[:, vcols])
        bv = np.ascontiguousarray(b_in[vcols].reshape(PAIRS, P).T)
        wout = np.ascontiguousarray(W_out[g * 512 : (g + 1) * 512, :])
        per_group[g] = dict(
            wqk=np.ascontiguousarray(wqk), bqk=bqk, wv=wv, bv=bv,
            wout=wout, tri=tri,
        )

    in_maps = []
    for c in range(8):
        b, g = c // 2, c % 2
        m = dict(per_group[g])
        m["xT"] = xTs[b]
        in_maps.append(m)
    return in_maps


def run(inputs, trace=False):
    if "nc" not in _CACHE:
        _CACHE["nc"] = _build_nc()
    nc = _CACHE["nc"]
    in_maps = _host_shards(**inputs)
    res = run_bass_kernel_spmd(
        nc, in_maps, core_ids=list(range(8)), trace=trace,
        trace_cores=list(range(8)) if trace else None,
    )
    b_out = np.asarray(inputs["b_out"], dtype=np.float32)
    out = np.empty((B, T, C), dtype=np.float32)
    for b in range(B):
        out[b] = res.results[2 * b]["out"] + res.results[2 * b + 1]["out"] + b_out
    return out, res


def kernel(**inputs) -> np.ndarray:
    out, _ = run(inputs, trace=False)
    return out



# revision 63
# speedup vs baseline: 1.3008x; 1.3008x over previous
"""Multi-head attention Trainium2 kernel (B=4, T=1024, C=1024, H=16, D=64).

Sharding over 8 NeuronCores: core c handles batch b = c//2 and head group
g = c%2 (heads [8g, 8g+8)).  Each core computes a partial out-projection
(its 8 heads' contribution, [T, C]); the host sums the two partials per
batch and adds b_out (plus the folded V-bias term bv @ W_out).  No
on-device collectives.

All matmul operands are bf16 (PSUM accumulation fp32); weights and x are
pre-packed on the host into SBUF layout so every DMA moves >=2KB
contiguous per partition.

Math (per core, bf16 matmuls, fp32 accumulation):
  XT[p, ki, t]  host-packed x[b].T
  QT/KT[f, t] = Wqk[:, f].T @ XT     (pair-stacked [128, T], Q pre-scaled
                                      1/8 on host, bias via ACT copy)
  V[t, f]     = XT-chunk.T @ Wv      (natural layout, ones col appended)
  S^T[k, q]   = KT-slice.T @ QT-slice  (per head, causal blocks only)
  P           = exp(S^T)  (ACT, bf16), tri mask on diagonal blocks (Pool)
  vals^T/s    = [V | 1].T @ P        (s = denominator in row 64)
  out[q, c]   = vals^T.T @ Wout-slice  (normalized via recip+mult chain)
"""

import os
import numpy as np
import ml_dtypes

import concourse.bass as bass
import concourse.mybir as mybir
import concourse.tile as tile
from concourse import bacc
from concourse.bass_utils import run_bass_kernel_spmd

B, T, C, H, D = 4, 1024, 1024, 16, 64
P = 128            # partitions
HPC = 8            # heads per core
PAIRS = 4          # head pairs per core
NK = C // P        # 8 contraction tiles
KT_TILES = T // P  # 8 k-tiles over sequence
QC = 512           # q-chunk (PSUM bank free size, fp32)
NQC = T // QC      # 2 q-chunks
F32 = mybir.dt.float32
BF16 = mybir.dt.bfloat16
AF = mybir.ActivationFunctionType
ALU = mybir.AluOpType

_CACHE = {}


def _build_nc():
    nc = bacc.Bacc(None, target_bir_lowering=False)

    xT = nc.dram_tensor("xT", [P, NK, T], BF16, kind="ExternalInput")
    wqk01 = nc.dram_tensor("wqk01", [P, 2, NK, P], BF16, kind="ExternalInput")
    wqk23 = nc.dram_tensor("wqk23", [P, 2, NK, P], BF16, kind="ExternalInput")
    wqk47 = nc.dram_tensor("wqk47", [P, 4, NK, P], BF16, kind="ExternalInput")
    wv = nc.dram_tensor("wv", [P, NK, HPC * D], BF16, kind="ExternalInput")
    wout = nc.dram_tensor("wout", [P, PAIRS, C], BF16, kind="ExternalInput")
    bqk = nc.dram_tensor("bqk", [P, 8], F32, kind="ExternalInput")
    tri2 = nc.dram_tensor("tri2", [P, 2, P], BF16, kind="ExternalInput")
    out = nc.dram_tensor("out", [T, C], BF16, kind="ExternalOutput")

    with tile.TileContext(nc) as tc:
        with (
            tc.tile_pool(name="consts", bufs=1) as consts,
            tc.tile_pool(name="wqk_p", bufs=1) as wqk_pool,
            tc.tile_pool(name="qkt", bufs=8) as qkt_pool,
            tc.tile_pool(name="vsb", bufs=8) as v_pool,
            tc.tile_pool(name="probs", bufs=48) as p_pool,
            tc.tile_pool(name="vals", bufs=8) as vals_pool,
            tc.tile_pool(name="smal", bufs=2) as s2_pool,
        ):
            # ---- warmup scratch (tiny memset so it has a writer) ----
            warm_sb = consts.tile([P, P], BF16)
            nc.vector.memset(warm_sb, 0.0)

            # ---- input DMAs (SP queue, arrival-critical order; few, large
            # transfers — HWDGE descriptor-gen is serialized at 625ns each)
            xt_ctx = tc.tile_pool(name="xt", bufs=1)
            xt_pool = xt_ctx.__enter__()
            w_all = wqk_pool.tile([P, 8, NK, P], BF16)
            nc.sync.dma_start(w_all[:, 0:2], wqk01[:, :, :, :])
            xt_sb = xt_pool.tile([P, NK, T], BF16)
            nc.sync.dma_start(xt_sb[:, 0, :], xT[:, 0, :])
            bqk_sb = consts.tile([P, 8], F32)
            nc.sync.dma_start(bqk_sb, bqk[:, :])
            tri2_sb = consts.tile([P, 2, P], BF16)
            nc.sync.dma_start(tri2_sb, tri2[:, :, :])
            for ki in range(1, NK):
                nc.sync.dma_start(xt_sb[:, ki, :], xT[:, ki, :])
            nc.sync.dma_start(w_all[:, 2:4], wqk23[:, :, :, :])
            nc.sync.dma_start(w_all[:, 4:8], wqk47[:, :, :, :])
            wv_sb = consts.tile([P, NK, HPC * D], BF16)
            nc.sync.dma_start(wv_sb, wv[:, :, :])
            wout_sb = consts.tile([P, PAIRS, C], BF16)
            nc.sync.dma_start(wout_sb, wout[:, :, :])

            # persistent v tiles; ones column memset early (no deps)
            v_sb = []
            for ti in range(KT_TILES):
                vt = v_pool.tile([P, HPC, D + 1], BF16, tag="v_sb", name=f"v{ti}")
                v_sb.append(vt)
                nc.vector.memset(vt[:, :, D : D + 1], 1.0)

            qkt_sb = {}

            # ---- phase 1 pools: QKV psum + scores psum = 4 + 4 banks ----
            p1_ctx = tc.tile_pool(name="qkv_ps", bufs=2, space="PSUM")
            p1 = p1_ctx.__enter__()
            p2_ctx = tc.tile_pool(name="s_ps", bufs=2, space="PSUM")
            p2 = p2_ctx.__enter__()

            def emit_warm_mms(ps, n):
                """Dead matmuls into the (not yet used) qc1 half of a slot's
                PSUM tile; the real qc1 accumulation's start flag clears it.
                Holds the PE p-state ramp while input DMAs land."""
                for _ in range(n):
                    nc.tensor.matmul(
                        ps[:, 1, 0:P], warm_sb[:, 0:P], warm_sb[:, 0:P],
                        start=True, stop=True, skip_group_check=True,
                    )

            _slot_ps = {}

            def emit_slot_half(slot, qc, warm=0):
                """Half of QT/KT slot: one q-chunk accumulation.  warm:
                interleave dead matmuls after each ki so the xt-DMA-paced
                start never leaves a PE gap (keeps the p-state ramp alive)."""
                if qc == 0 and slot not in _slot_ps:
                    _slot_ps[slot] = p1.tile(
                        [P, NQC, QC], F32, tag="qkv", name=f"qkvps{slot}"
                    )
                ps = _slot_ps[slot]
                for ki in range(NK):
                    nc.tensor.matmul(
                        ps[:, qc, :],
                        w_all[:, slot, ki, :],
                        xt_sb[:, ki, qc * QC : (qc + 1) * QC],
                        start=(ki == 0),
                        stop=(ki == NK - 1),
                    )
                    if warm and qc == 0:
                        emit_warm_mms(ps, warm)
                if qc == NQC - 1:
                    dst = qkt_pool.tile([P, T], BF16, tag="qkt", name=f"qkt{slot}")
                    qkt_sb[slot] = dst
                    nc.vector.tensor_scalar_add(
                        dst,
                        ps.rearrange("p a b -> p (a b)"),
                        bqk_sb[:, slot : slot + 1],
                    )

            def emit_v(ti):
                ps = p1.tile([P, QC], F32, tag="qkv", name=f"vps{ti}")
                for ki in range(NK):
                    nc.tensor.matmul(
                        ps,
                        xt_sb[:, ki, ti * P : (ti + 1) * P],
                        wv_sb[:, ki, :],
                        start=(ki == 0),
                        stop=(ki == NK - 1),
                    )
                nc.vector.tensor_copy(
                    v_sb[ti][:, :, 0:D], ps.rearrange("p (h d) -> p h d", h=HPC)
                )

            p_tiles = {}  # (pair, qc, kj) -> P tile [128, 2, QC] bf16

            def emit_sc(pair, qc, kjs):
                """Score tiles + exp (+ tri mask on diagonal blocks)."""
                qt = qkt_sb[2 * pair]
                kt = qkt_sb[2 * pair + 1]
                for kj in kjs:
                    j0 = kj - 4 * qc
                    q_lo = max(j0, 0) * P
                    sps = p2.tile([P, 2, QC], F32, tag="s", name="sps")
                    for hl in range(2):
                        d0 = D * hl
                        nc.tensor.matmul(
                            sps[:, hl, q_lo:QC],
                            kt[d0 : d0 + D, kj * P : (kj + 1) * P],
                            qt[d0 : d0 + D, qc * QC + q_lo : (qc + 1) * QC],
                            start=True,
                            stop=True,
                        )
                    pt = p_pool.tile([P, 2, QC], BF16, tag="probs")
                    p_tiles[(pair, qc, kj)] = pt
                    nc.scalar.activation(
                        pt[:, :, q_lo:QC], sps[:, :, q_lo:QC], AF.Exp
                    )
                    if j0 >= 0:
                        nc.vector.tensor_tensor(
                            pt[:, :, q_lo : q_lo + P],
                            pt[:, :, q_lo : q_lo + P],
                            tri2_sb[:, :, :],
                            ALU.mult,
                        )

            vals_sb = {}

            def emit_av(pair, qc, p3, drain="act_vun"):
                """attnV for both heads + normalize chain: s (row 64) ->
                s2 (parallel DVE/ACT copies) -> partition-broadcast (Pool)
                -> divide into vals (bf16)."""
                n_kt = 4 * (qc + 1)
                vv = p3.tile([P, 2, QC], F32, tag="vv", name=f"vv{pair}_{qc}")
                for hl in range(2):
                    h_abs = 2 * pair + hl
                    for kj in range(n_kt):
                        j0 = kj - 4 * qc
                        q_lo = max(j0, 0) * P
                        nc.tensor.matmul(
                            vv[0 : D + 1, hl, q_lo:QC],
                            v_sb[kj][:, h_abs, :],
                            p_tiles[(pair, qc, kj)][:, hl, q_lo:QC],
                            start=(kj == 0),
                            stop=(kj == n_kt - 1),
                            skip_group_check=True,
                        )
                # Drain PSUM immediately (unnormalized vals on DVE, s-row on
                # ACT) so the vv ring never waits on the normalize chain;
                # the recip/broadcast/scale then runs SBUF-only off the
                # critical path, gating only the out-projection.
                key = (pair, qc)
                vun = vun_pool.tile(
                    [D, 2, QC], BF16, tag="vun", name=f"vun{pair}_{qc}"
                )
                s2 = s2_pool.tile([1, 2 * QC], F32, tag="s2")
                if drain == "act_vun":
                    nc.scalar.activation(vun, vv[0:D, :, :], AF.Copy)
                    nc.vector.tensor_copy(
                        s2[0:1, :],
                        vv[D : D + 1, :, :].rearrange("p a b -> p (a b)"),
                    )
                else:
                    nc.vector.tensor_copy(vun, vv[0:D, :, :])
                    nc.scalar.activation(
                        s2[0:1, :],
                        vv[D : D + 1, :, :].rearrange("p a b -> p (a b)"),
                        AF.Copy,
                    )
                r2 = s2_pool.tile([1, 2 * QC], F32, tag="r2")
                nc.vector.reciprocal_approx_fast(r2, s2)
                rbc = rbc_pool.tile([D, 2 * QC], F32, tag="rbc")
                nc.gpsimd.partition_broadcast(rbc[:, 0:QC], r2[0:1, 0:QC])
                nc.gpsimd.partition_broadcast(rbc[:, QC:], r2[0:1, QC:])
                vals_sb[key] = vals_pool.tile(
                    [P, QC], BF16, tag="vals", name=f"vals{pair}_{qc}"
                )
                nc.vector.tensor_tensor(
                    vals_sb[key][0:D, :], vun[:, 0, :], rbc[:, 0:QC], ALU.mult
                )
                nc.gpsimd.tensor_tensor(
                    vals_sb[key][D : 2 * D, :], vun[:, 1, :], rbc[:, QC:],
                    ALU.mult,
                )

            _oq_ps = {}

            def emit_oq(qc, tsub, cc, p4, pairs, copy_eng):
                """Out-projection chunk [128 q, 512 c]; `pairs` may split the
                accumulation across calls (last call finishes + stores)."""
                q0 = tsub * P
                key = (qc, tsub, cc)
                if key not in _oq_ps:
                    _oq_ps[key] = p4.tile(
                        [P, QC], F32, tag="ops", name=f"ops{qc}_{tsub}_{cc}"
                    )
                ops = _oq_ps[key]
                for pair in pairs:
                    nc.tensor.matmul(
                        ops,
                        vals_sb[(pair, qc)][:, q0 : q0 + P],
                        wout_sb[:, pair, cc * QC : (cc + 1) * QC],
                        start=(pair == 0),
                        stop=(pair == PAIRS - 1),
                        skip_group_check=True,
                    )
                if pairs[-1] != PAIRS - 1:
                    return
                o_sb = out_pool.tile([P, QC], BF16, tag="o_sb")
                if copy_eng == "act":
                    nc.scalar.activation(o_sb, ops, AF.Copy)
                else:
                    nc.vector.tensor_copy(o_sb, ops)
                nc.sync.dma_start(
                    out[qc * QC + q0 : qc * QC + q0 + P, cc * QC : (cc + 1) * QC],
                    o_sb,
                )

            # ---- phase 1: QKV + V matmuls with all score tiles woven in.
            # Weave rule: ~1 score tile (0.85us PE, 1us ACT exp) per ~1.2us
            # of independent filler matmuls, so the 2-deep score-PSUM ring
            # never stalls the in-order PE queue.  qc0 pairs early (attnV
            # consumes them first in phase 2), pair 3 qc1 last.
            _slot_ps[0] = p1.tile([P, NQC, QC], F32, tag="qkv", name="qkvps0")
            emit_warm_mms(_slot_ps[0], 34)
            emit_slot_half(0, 0, warm=2)
            emit_slot_half(1, 0, warm=2)
            emit_slot_half(0, 1)
            emit_slot_half(1, 1)
            emit_slot_half(2, 0)
            emit_sc(0, 0, [0, 1, 2])
            emit_slot_half(2, 1)
            emit_sc(0, 0, [3])
            emit_sc(0, 1, [0, 1])
            emit_slot_half(3, 0)
            emit_sc(0, 1, [2, 3, 4])
            emit_slot_half(3, 1)
            emit_sc(0, 1, [5, 6, 7])
            emit_slot_half(4, 0)
            emit_sc(1, 0, [0, 1, 2])
            emit_slot_half(4, 1)
            emit_sc(1, 0, [3])
            emit_sc(1, 1, [0, 1])
            emit_slot_half(5, 0)
            emit_sc(1, 1, [2, 3, 4])
            emit_slot_half(5, 1)
            emit_sc(1, 1, [5, 6, 7])
            emit_v(0)
            emit_sc(2, 0, [0])
            emit_v(1)
            emit_sc(2, 0, [1, 2])
            emit_v(2)
            emit_sc(2, 0, [3])
            emit_v(3)
            emit_sc(2, 1, [0, 1])
            emit_slot_half(6, 0)
            emit_sc(2, 1, [2, 3, 4])
            emit_slot_half(6, 1)
            emit_sc(2, 1, [5, 6, 7])
            emit_slot_half(7, 0)
            emit_slot_half(7, 1)
            emit_sc(3, 0, [0, 1, 2])
            emit_v(4)
            emit_sc(3, 0, [3])
            emit_sc(3, 1, [0, 1])
            emit_v(5)
            emit_sc(3, 1, [2, 3])
            emit_sc(3, 1, [4, 5])
            emit_v(6)
            emit_sc(3, 1, [6, 7])
            emit_v(7)

            p2_ctx.__exit__(None, None, None)
            p1_ctx.__exit__(None, None, None)
            xt_ctx.__exit__(None, None, None)
            vun_ctx = tc.tile_pool(name="vun", bufs=3)
            vun_pool = vun_ctx.__enter__()
            rbc_ctx = tc.tile_pool(name="rbc", bufs=2)
            rbc_pool = rbc_ctx.__enter__()
            outs_ctx = tc.tile_pool(name="outs", bufs=3)
            out_pool = outs_ctx.__enter__()

            # ---- phase 2: attnV blocks (3-deep 2-bank PSUM ring) with the
            # out-projection pool nested alongside (6 + 2 banks), so oq
            # chunks fill PE while normalize chains drain ----
            p3_ctx = tc.tile_pool(name="v_ps", bufs=3, space="PSUM")
            p3 = p3_ctx.__enter__()
            p4_ctx = tc.tile_pool(name="o_ps", bufs=2, space="PSUM")
            p4 = p4_ctx.__enter__()

            emit_av(0, 0, p3)
            emit_av(1, 0, p3)
            emit_av(3, 0, p3)
            emit_av(2, 0, p3)
            emit_av(0, 1, p3)
            emit_av(1, 1, p3)
            emit_av(2, 1, p3)
            emit_oq(0, 0, 0, p4, [0, 1, 2, 3], "act")
            emit_oq(0, 0, 1, p4, [0, 1, 2, 3], "act")
            emit_av(3, 1, p3)
            for tsub in range(1, 4):
                for cc in range(2):
                    emit_oq(0, tsub, cc, p4, [0, 1, 2, 3], "act")
            for tsub in range(4):
                for cc in range(2):
                    emit_oq(1, tsub, cc, p4, [0, 1, 2, 3], "act")

            p4_ctx.__exit__(None, None, None)
            p3_ctx.__exit__(None, None, None)
            outs_ctx.__exit__(None, None, None)
            rbc_ctx.__exit__(None, None, None)
            vun_ctx.__exit__(None, None, None)

    nc.compile()
    return nc


def _host_shards(x, mask, W_in, b_in, W_out, b_out):
    """Build the 8 per-core input maps (bf16, SBUF-packed layouts)."""
    del mask  # causal structure is hardcoded (tri2 built locally)
    x = np.asarray(x, dtype=np.float32)
    W_in = np.asarray(W_in, dtype=np.float32)
    b_in = np.asarray(b_in, dtype=np.float32)
    W_out = np.asarray(W_out, dtype=np.float32)
    bf = ml_dtypes.bfloat16

    tri = np.triu(np.ones((P, P), dtype=np.float32))  # tri[k, q] = 1 if k <= q
    tri2 = np.ascontiguousarray(
        np.broadcast_to(tri[:, None, :], (P, 2, P))
    ).astype(bf)
    xTs = [
        np.ascontiguousarray(
            x[b].T.reshape(NK, P, T).transpose(1, 0, 2)
        ).astype(bf)
        for b in range(B)
    ]

    per_group = {}
    for g in range(2):
        wqk = np.empty((8, P, NK, P), dtype=np.float32)
        bqk = np.empty((P, 8), dtype=np.float32)
        for p in range(PAIRS):
            qcols = slice((8 * g + 2 * p) * D, (8 * g + 2 * p + 2) * D)
            kcols = slice(C + (8 * g + 2 * p) * D, C + (8 * g + 2 * p + 2) * D)
            wqk[2 * p] = (
                W_in[:, qcols].reshape(NK, P, P).transpose(1, 0, 2) * 0.125
            )
            wqk[2 * p + 1] = W_in[:, kcols].reshape(NK, P, P).transpose(1, 0, 2)
            bqk[:, 2 * p] = b_in[qcols] * 0.125
            bqk[:, 2 * p + 1] = b_in[kcols]
        vcols = slice(2 * C + g * 512, 2 * C + (g + 1) * 512)
        wv = np.ascontiguousarray(
            W_in[:, vcols].reshape(NK, P, 512).transpose(1, 0, 2)
        ).astype(bf)
        wout = np.ascontiguousarray(
            W_out[g * 512 : (g + 1) * 512, :]
            .reshape(PAIRS, P, C)
            .transpose(1, 0, 2)
        ).astype(bf)
        wqk_p = wqk.transpose(1, 0, 2, 3)  # [p, slot, ki, f]
        per_group[g] = dict(
            wqk01=np.ascontiguousarray(wqk_p[:, 0:2]).astype(bf),
            wqk23=np.ascontiguousarray(wqk_p[:, 2:4]).astype(bf),
            wqk47=np.ascontiguousarray(wqk_p[:, 4:8]).astype(bf),
            bqk=bqk, wv=wv, wout=wout, tri2=tri2,
        )

    in_maps = []
    for c in range(8):
        b, g = c // 2, c % 2
        m = dict(per_group[g])
        m["xT"] = xTs[b]
        in_maps.append(m)
    return in_maps


def run(inputs, trace=False):
    if "nc" not in _CACHE:
        _CACHE["nc"] = _build_nc()
    nc = _CACHE["nc"]
    in_maps = _host_shards(**inputs)
    res = run_bass_kernel_spmd(
        nc, in_maps, core_ids=list(range(8)), trace=trace,
        trace_cores=list(range(8)) if trace else None,
    )
    b_in = np.asarray(inputs["b_in"], dtype=np.float32)
    W_out = np.asarray(inputs["W_out"], dtype=np.float32)
    b_out = np.asarray(inputs["b_out"], dtype=np.float32)
    # V-bias folded out of the device kernel: vals_true = vals_dev + b_v,
    # so out_true = out_dev + b_v @ W_out (+ b_out), added once per batch.
    bias = b_out + b_in[2 * C :] @ W_out
    out = np.empty((B, T, C), dtype=np.float32)
    for b in range(B):
        out[b] = (
            np.asarray(res.results[2 * b]["out"], dtype=np.float32)
            + np.asarray(res.results[2 * b + 1]["out"], dtype=np.float32)
            + bias
        )
    return out, res


def kernel(**inputs) -> np.ndarray:
    out, _ = run(inputs, trace=False)
    return out
